# revision 8
# baseline (speedup 1.0000x reference)
"""Category-equality Gram matrix kernel for TRN2.

out[i, j] = 1.0 if Z[i] == Z[j] else 0.0, Z: [16384] int32 labels in [0, 64).

Row-parallel across 8 NeuronCores: core i computes rows [i*2048, (i+1)*2048).

The reference computes one_hot(Z) @ one_hot(Z).T. This kernel computes the
same matmul on the TensorEngine, but with the left one-hot rows pre-weighted
by powers of two so each PSUM f32 value packs 16 consecutive output rows as
an exact integer in [0, 65535]:

    P[p, j] = sum_{k=0..15} 2^k * [Z[base + 16p + k] == Z[j]]

K = 64 classes only fills half the 128-deep PE array, so the stationary
operand stacks the low-byte weights (2^0..2^7, values <= 255, bf16-exact)
in contraction rows 0-63 and the high-byte weights (2^8..2^15, values =
mask*256, also bf16-exact) in rows 64-127, against a partition-replicated
one-hot rhs (fp8 e4m3: 0/1 exact, halves load bytes). One matmul of
[K=128, M=128] x [128, 512] per 512-column tile, no accumulation passes.

PSUM -> SBUF evacuation casts f32 -> uint16 (exact for these integers) in
1024-column tiles, alternating VectorE / ScalarE so both PSUM-capable
engines drain banks in parallel. Each core stores a [128, 16384] uint16
tile = 4 MiB instead of the 128 MiB f32 block: a 16x denser exact encoding
of the same Gram matrix, decoded on the host by bit-plane extraction.

Trace-verified details: dummy matmuls on a zeroed tile warm the PE HAM
clock gate (1.2 -> 2.4 GHz) during the load window; the first one-hot load
chunk is small so the first real matmul's DMA-receipt wait ends early; load
enqueues alternate HWDGE rings (each dma_start occupies its ring ~0.7 us);
the final store is split across both rings to shorten the drain tail.
"""

import ml_dtypes
import numpy as np

import concourse.tile as tile
from concourse import bacc, mybir
from concourse.bass_utils import run_bass_kernel_spmd

N = 16384          # number of labels / output dim
M = 8              # cores
RPC = N // M       # 2048 rows per core
CLS = 64           # label classes
G = 16             # output rows packed per uint16
P = 128            # SBUF partitions (= packed rows per core: RPC / G)
NT = 512           # matmul free-dim tile (one PSUM bank of f32)
ST = 1024          # super-tile: psum tile / copy width (2 banks)
NSUP = N // ST     # 16
SCHUNK = 2048      # store chunk (cols)
# staggered one-hot load chunks: small first so matmuls start early
LCHUNKS = [512, 1536, 2048, 4096, 4096, 4096]
assert sum(LCHUNKS) == N
N_WARM = 36        # dummy matmuls to warm the PE clock gate (~3.5 us)

OH_DT = mybir.dt.float8e4
OH_NP = ml_dtypes.float8_e4m3

_NC_CACHE = None


def _build_nc():
    nc = bacc.Bacc("TRN2", target_bir_lowering=False, debug=False, num_devices=M)
    # w2[0:64, p]  = sum_{k<8} 2^k     * [Z[base+16p+k] == c]   (c = row)
    # w2[64:, p]   = sum_{k<8} 2^(k+8) * [Z[base+16p+8+k] == c]
    w2 = nc.dram_tensor("w2", [P, P], mybir.dt.bfloat16, kind="ExternalInput").ap()
    # oh2[c, j] = oh2[c + 64, j] = [Z[j] == c]
    oh2 = nc.dram_tensor("oh2", [P, N], OH_DT, kind="ExternalInput").ap()
    out = nc.dram_tensor("out", [P, N], mybir.dt.uint16, kind="ExternalOutput").ap()

    with tile.TileContext(nc) as tc:
        with tc.tile_pool(name="wp", bufs=1) as wp, \
             tc.tile_pool(name="zp", bufs=1) as zp, \
             tc.tile_pool(name="ohp", bufs=1) as ohp, \
             tc.tile_pool(name="op", bufs=1) as op, \
             tc.tile_pool(name="wpp", bufs=1, space="PSUM") as wpp, \
             tc.tile_pool(name="pp", bufs=3, space="PSUM") as pp:
            # PE warm-up: matmuls on a zeroed tile, issued before any DMA
            # dependency, keep the PE busy through the HAM activity window so
            # the real matmuls run at 2.4 GHz instead of 1.2 GHz.
            wz = zp.tile([P, P], mybir.dt.bfloat16)
            nc.gpsimd.memset(wz[:], 0.0)
            pw = wpp.tile([P, NT], mybir.dt.float32)
            for _ in range(N_WARM):
                nc.tensor.matmul(pw[:, 0:P], wz[:], wz[:, 0:P],
                                 start=True, stop=True)

            w2s = wp.tile([P, P], mybir.dt.bfloat16)
            nc.sync.dma_start(w2s[:], w2[:, :])
            oh2s = ohp.tile([P, N], OH_DT)
            # alternate load enqueues across both HWDGE rings: each enqueue
            # occupies its issuing engine ~0.7 us
            c0 = 0
            for ci, w in enumerate(LCHUNKS):
                ring = nc.scalar if ci % 2 == 0 else nc.sync
                ring.dma_start(oh2s[:, c0:c0 + w], oh2[:, c0:c0 + w])
                c0 += w
            outs = op.tile([P, N], mybir.dt.uint16)

            rings = [nc.sync, nc.scalar]
            n_store = 0
            for t in range(NSUP):
                ps = pp.tile([P, ST], mybir.dt.float32)
                for h in range(ST // NT):
                    j0 = t * ST + h * NT
                    nc.tensor.matmul(
                        ps[:, h * NT:(h + 1) * NT], w2s[:],
                        oh2s[:, j0:j0 + NT],
                        start=True, stop=True,
                    )
                dst = outs[:, t * ST:(t + 1) * ST]
                if t % 2 == 0:
                    nc.vector.tensor_copy(dst, ps[:])
                else:
                    nc.scalar.activation(
                        dst, ps[:], mybir.ActivationFunctionType.Copy
                    )
                c1 = (t + 1) * ST
                if t == NSUP - 1:
                    # final 2048 cols: two 1024-col stores on both rings so
                    # the tail drains in parallel
                    nc.sync.dma_start(out[:, c1 - SCHUNK:c1 - ST],
                                      outs[:, c1 - SCHUNK:c1 - ST])
                    nc.scalar.dma_start(out[:, c1 - ST:c1], outs[:, c1 - ST:c1])
                elif c1 % SCHUNK == 0:
                    ring = rings[n_store % 2]
                    n_store += 1
                    ring.dma_start(out[:, c1 - SCHUNK:c1],
                                   outs[:, c1 - SCHUNK:c1])
    nc.compile()
    return nc


def _get_nc():
    global _NC_CACHE
    if _NC_CACHE is None:
        _NC_CACHE = _build_nc()
    return _NC_CACHE


def _in_maps(Z: np.ndarray) -> list[dict[str, np.ndarray]]:
    z = np.asarray(Z).reshape(-1).astype(np.int32)
    ohr = z[None, :] == np.arange(CLS, dtype=np.int32)[:, None]   # [64, N]
    oh2 = np.ascontiguousarray(
        np.concatenate([ohr, ohr], axis=0)
    ).astype(OH_NP)                                               # [128, N]
    pow_lo = (2.0 ** np.arange(8)).astype(np.float64)
    pow_hi = (2.0 ** np.arange(8, 16)).astype(np.float64)
    maps = []
    for i in range(M):
        lab = z[i * RPC:(i + 1) * RPC].reshape(P, G)              # [p, k]
        ohl = lab[:, :, None] == np.arange(CLS, dtype=np.int32)   # [p, k, c]
        wlo = np.einsum("pkc,k->cp", ohl[:, :8, :], pow_lo)       # <= 255
        whi = np.einsum("pkc,k->cp", ohl[:, 8:, :], pow_hi)       # mask * 256
        w2_i = np.ascontiguousarray(
            np.concatenate([wlo, whi], axis=0)
        ).astype(ml_dtypes.bfloat16)                              # [128, 128]
        maps.append({"w2": w2_i, "oh2": oh2})
    return maps


def kernel(Z: np.ndarray, **_ignored) -> np.ndarray:
    Z = np.asarray(Z).reshape(-1)
    assert Z.shape == (N,), Z.shape
    nc = _get_nc()
    res = run_bass_kernel_spmd(nc, _in_maps(Z), list(range(M)))
    out = np.empty((N, N), dtype=np.float32)
    o3 = out.reshape(M * P, G, N)
    for i in range(M):
        packed = res.results[i]["out"]                            # [128, N] u16
        for b in range(G):
            o3[i * P:(i + 1) * P, b, :] = (packed >> b) & 1
    return out
